# revision 39
# baseline (speedup 1.0000x reference)
"""CondConv (routing -> per-sample mixed 3x3 conv -> frozen BN -> ReLU -> residual)
on 8 Trainium2 NeuronCores, data-parallel over batch (4 samples/core).

Per core:
  - expert bank resident in SBUF as bf16, host-pretransposed to
    [cin, cout-half, kk, 128] so each cout half is contiguous
  - routing: GAP (DVE reduce) -> dot with route_w (DVE + gpsimd partition
    all-reduce; keeps the PE queue free for conv matmuls) -> sigmoid (ACT)
  - per-sample mixed kernel: DVE scalar_tensor_tensor accumulation in bf16,
    split per cout half so the first conv starts after half the mixing
  - conv: per output tile, 18 accumulating bf16 matmuls (2 cin tiles x 3x3
    taps; fp32 PSUM) against width-padded bf16 images; moving dim = 8 rows
    x 56 cols = 448; bf16 weight loads get FWL so LDW hides under the stream
  - BN(frozen)+ReLU fused into the ACT PSUM evacuation, residual add on DVE,
    fp32 output
"""

import threading

import ml_dtypes
import numpy as np

import concourse.bass as bass
import concourse.mybir as mybir
import concourse.tile as tile
from concourse import bacc, bass_isa
from concourse.bass_utils import run_bass_kernel_spmd

F32 = mybir.dt.float32
BF16 = mybir.dt.bfloat16
AX = mybir.AxisListType
OP = mybir.AluOpType
AF = mybir.ActivationFunctionType

N_CORES = 8
B, CIN, COUT, H, W, KS, E = 32, 256, 256, 56, 56, 3, 4
BPC = B // N_CORES  # samples per core
CT = CIN // 128     # cin partition tiles
OTN = COUT // 128   # cout partition tiles
KK = KS * KS
WP = W + 2          # width zero-padded (kj shifts); height handled by clipping
RC = 7              # row chunks per image
RH = H // RC        # rows per chunk
NF = RH * W         # moving-dim elements per matmul
BN_EPS = 1e-5

# conv taps, center first: the center tap covers the full output chunk, so it
# carries start=True and clears every PSUM has_written bit; row-clipped taps
# then accumulate flat sub-slices (= 'same' padding semantics at top/bottom).
TAPS = [(1, 1)] + [(ki, kj) for ki in range(KS) for kj in range(KS)
                   if (ki, kj) != (1, 1)]


def build_bass():
    nc = bacc.Bacc("TRN2", target_bir_lowering=False, debug=False)

    x_d = nc.dram_tensor("x", [BPC, CIN, H, WP], BF16, kind="ExternalInput")
    wt_d = nc.dram_tensor("wt", [E, CIN, OTN, KK, 128], BF16,
                          kind="ExternalInput")
    rwt_d = nc.dram_tensor("rwt", [CIN, E], F32, kind="ExternalInput")
    rb_d = nc.dram_tensor("rb", [E], F32, kind="ExternalInput")
    bnp_d = nc.dram_tensor("bnp", [COUT, 4], F32, kind="ExternalInput")
    y_d = nc.dram_tensor("y", [BPC, COUT, H, W], F32, kind="ExternalOutput")

    x_ap = x_d.ap()
    wt_ap = wt_d.ap()
    rwt_ap = rwt_d.ap()
    rb_ap = rb_d.ap()
    bnp_ap = bnp_d.ap()
    y_ap = y_d.ap()

    with tile.TileContext(nc) as tc:
        with (
            tc.tile_pool(name="wbp", bufs=1) as wbp,
            tc.tile_pool(name="xpp", bufs=1) as xpp,
            tc.tile_pool(name="mwp", bufs=1) as mwp,
            tc.tile_pool(name="otp", bufs=10) as otp,
            tc.tile_pool(name="snp", bufs=1) as snp,
            tc.tile_pool(name="smp", bufs=4) as smp,
            tc.tile_pool(name="psp", bufs=6, space="PSUM") as psp,
        ):
            # ---- persistent tiles ----
            wb = [[wbp.tile([128, OTN, KK, 128], BF16, name=f"wb{e}_{t}",
                            tag=f"wb{e}_{t}")
                   for t in range(CT)] for e in range(E)]
            xp = [[xpp.tile([128, H, WP], BF16, name=f"xp{i}_{t}",
                            tag=f"xp{i}_{t}")
                   for t in range(CT)] for i in range(2)]
            mw = [[mwp.tile([128, OTN, KK, 128], BF16, name=f"mw{i}_{t}",
                            tag=f"mw{i}_{t}")
                   for t in range(CT)] for i in range(2)]
            rwt_sb = [snp.tile([128, E], F32, name=f"rwt{t}", tag=f"rwt{t}")
                      for t in range(CT)]
            rb_bc = snp.tile([128, E], F32, name="rb_bc", tag="rb_bc")
            bn_sb = [snp.tile([128, 4], F32, name=f"bn{o}", tag=f"bn{o}")
                     for o in range(OTN)]
            bn_inv = [snp.tile([128, 1], F32, name=f"bninv{o}", tag=f"bninv{o}")
                      for o in range(OTN)]
            bn_shift = [snp.tile([128, 1], F32, name=f"bnsh{o}", tag=f"bnsh{o}")
                        for o in range(OTN)]

            # ---- preamble DMAs in priority order: queue order = bandwidth
            # priority. x(0) tiles split across two queues (routing critical
            # path), tiny params next, then the expert bank with the oi=0
            # halves first (mixing consumes them first).
            nc.sync.dma_start(out=xp[0][0], in_=x_ap[0, 0:128, :, :])
            nc.scalar.dma_start(out=xp[0][1], in_=x_ap[0, 128:256, :, :])
            # tiny params on the otherwise-idle gpsimd queue so they neither
            # occupy the sync queue head nor delay ACT's compute stream
            for t in range(CT):
                nc.gpsimd.dma_start(out=rwt_sb[t],
                                    in_=rwt_ap[t * 128:(t + 1) * 128, :])
            nc.gpsimd.dma_start(
                out=rb_bc,
                in_=bass.AP(tensor=rb_ap.tensor, offset=0, ap=[[0, 128], [1, E]]))
            for o in range(OTN):
                nc.gpsimd.dma_start(out=bn_sb[o],
                                    in_=bnp_ap[o * 128:(o + 1) * 128, :])
            # expert bank: oi0 halves gate the first mix — split them between
            # sync (e0,e1) and gpsimd (e2,e3); oi1 halves follow on sync
            for oi in range(OTN):
                for e in range(E):
                    for t in range(CT):
                        eng = nc.gpsimd if (oi == 0 and e >= 2) else nc.sync
                        eng.dma_start(out=wb[e][t][:, oi],
                                      in_=wt_ap[e, t * 128:(t + 1) * 128, oi])

            # all-zeros per-partition scalar: explicit AP bias for ACT funcs
            # (the float-bias path needs a pre-registered const-AP database)
            zeros1 = snp.tile([128, 1], F32, name="zeros1", tag="zeros1")
            nc.vector.memset(zeros1, 0.0)

            # BN folded scale/shift: inv = gamma / sqrt(var+eps);
            # shift = beta - mean * inv
            for o in range(OTN):
                ve = smp.tile([128, 1], F32, name=f"ve{o}", tag="ve")
                nc.vector.tensor_scalar_add(ve, bn_sb[o][:, 3:4], BN_EPS)
                sq = smp.tile([128, 1], F32, name=f"sq{o}", tag="sq")
                nc.scalar.activation(out=sq, in_=ve, func=AF.Sqrt, bias=zeros1)
                nc.vector.reciprocal(out=bn_inv[o], in_=sq)
                nc.vector.tensor_mul(bn_inv[o], bn_inv[o], bn_sb[o][:, 0:1])
                mi = smp.tile([128, 1], F32, name=f"mi{o}", tag="mi")
                nc.vector.tensor_mul(mi, bn_sb[o][:, 2:3], bn_inv[o])
                nc.vector.tensor_sub(bn_shift[o], bn_sb[o][:, 1:2], mi)

            # scratch target for the ACT-side pooled copy (only accum_out used)
            pscr = snp.tile([128, H * W // 2], BF16, name="pscr", tag="pscr")

            # warm-up operands: dependency-gated dummy matmuls keep the PE
            # HAM window busy right before the first real matmul so the real
            # stream starts at full clock (warm_x is touched from `prod` in
            # prep(0) to time the dummies against the routing chain)
            warm_w = snp.tile([128, 128], BF16, name="warm_w", tag="warm_w")
            nc.vector.memset(warm_w, 0.0)
            warm_x = snp.tile([128, NF], BF16, name="warm_x", tag="warm_x")
            nc.vector.memset(warm_x, 0.0)

            def prep(s):
                """Routing + weight mixing for sample s (no PE involvement)."""
                i = s % 2
                pooled = [smp.tile([128, 1], F32, name=f"pool{s}_{t}",
                                   tag=f"pool{t}") for t in range(CT)]
                ph = smp.tile([128, 1], F32, name=f"ph{s}", tag="ph")
                # GAP: tile 0 on DVE; tile 1 split into a DVE half and an ACT
                # (Copy + accum_out) half so its reduction finishes ~2x sooner
                # after the tile-1 DMA lands
                nc.vector.reduce_sum(out=pooled[0], in_=xp[i][0][:, :, 1:W + 1],
                                     axis=AX.XY)
                nc.vector.reduce_sum(out=pooled[1],
                                     in_=xp[i][1][:, 0:H // 2, 1:W + 1],
                                     axis=AX.XY)
                nc.scalar.activation(out=pscr, in_=xp[i][1][:, H // 2:H, 1:W + 1],
                                     func=AF.Copy, accum_out=ph)
                prod = smp.tile([128, E], F32, name=f"prod{s}", tag="prod")
                nc.vector.tensor_scalar_mul(prod, rwt_sb[0], pooled[0])
                nc.vector.scalar_tensor_tensor(out=prod, in0=rwt_sb[1],
                                               scalar=pooled[1], in1=prod,
                                               op0=OP.mult, op1=OP.add)
                nc.vector.scalar_tensor_tensor(out=prod, in0=rwt_sb[1],
                                               scalar=ph, in1=prod,
                                               op0=OP.mult, op1=OP.add)
                if s == 0:
                    # touch warm_x from prod, then issue the warm-up matmuls:
                    # they run while the routing tail + mixing completes
                    nc.vector.tensor_copy(warm_x[0:1, 0:E], prod[0:1, :])
                    wps = psp.tile([128, NF], F32, name="warm_ps",
                                   tag="warmps", bufs=1)
                    for _ in range(18):
                        nc.tensor.matmul(wps[:], lhsT=warm_w, rhs=warm_x,
                                         start=True, stop=True)
                lg = smp.tile([128, E], F32, name=f"lg{s}", tag="lg")
                nc.gpsimd.partition_all_reduce(lg, prod, channels=128,
                                               reduce_op=bass_isa.ReduceOp.add)
                nc.vector.scalar_tensor_tensor(out=lg, in0=lg,
                                               scalar=1.0 / (H * W), in1=rb_bc,
                                               op0=OP.mult, op1=OP.add)
                rr = smp.tile([128, E], F32, name=f"rr{s}", tag="rr")
                nc.scalar.activation(out=rr, in_=lg, func=AF.Sigmoid, bias=zeros1)
                # mix per cout half: the first conv of the sample only waits
                # for the oi=0 half of the bank. cin tile 0 accumulates on
                # DVE; tile 1 gets its expert scaling from ACT (scaled Copy)
                # with DVE doing only the adds, so the two chains overlap.
                for oi in range(OTN):
                    nc.vector.tensor_scalar_mul(mw[i][0][:, oi],
                                                wb[0][0][:, oi], rr[:, 0:1])
                    for e in range(1, E):
                        nc.vector.scalar_tensor_tensor(
                            out=mw[i][0][:, oi], in0=wb[e][0][:, oi],
                            scalar=rr[:, e:e + 1], in1=mw[i][0][:, oi],
                            op0=OP.mult, op1=OP.add)
                    ce = [smp.tile([128, KK, 128], BF16, name=f"ce{s}_{oi}_{e}",
                                   tag=f"ce{e}", bufs=2) for e in range(E)]
                    for e in range(E):
                        nc.scalar.activation(out=ce[e], in_=wb[e][1][:, oi],
                                             func=AF.Copy, scale=rr[:, e:e + 1])
                    nc.vector.tensor_add(mw[i][1][:, oi], ce[0], ce[1])
                    nc.vector.tensor_add(mw[i][1][:, oi], mw[i][1][:, oi], ce[2])
                    nc.vector.tensor_add(mw[i][1][:, oi], mw[i][1][:, oi], ce[3])

            def conv(s, oi):
                """One output channel tile of sample s: matmuls + BN/ReLU +
                residual + store."""
                i = s % 2
                o0 = oi * 128
                n_mm = len(TAPS) * CT
                for rc in range(RC):
                    r0 = rc * RH
                    acc = psp.tile([128, NF], F32, name=f"acc{s}_{oi}_{rc}",
                                   tag="acc")
                    k = 0
                    for t in range(CT):
                        # t-major so a chunk's first 9 matmuls only need the
                        # cin-tile-0 mix chain (shaves the sample-0 start)
                        for ki, kj in TAPS:
                            # valid output rows for this tap (clipped at top/
                            # bottom; kj handled by the zero-padded columns)
                            h_lo = max(r0, 1 - ki)
                            h_hi = min(r0 + RH - 1, H - ki)
                            kki = ki * KS + kj
                            nc.tensor.matmul(
                                acc[:, (h_lo - r0) * W:(h_hi - r0 + 1) * W],
                                lhsT=mw[i][t][:, oi, kki, :],
                                rhs=xp[i][t][:, h_lo + ki - 1:h_hi + ki,
                                             kj:kj + W],
                                start=(k == 0), stop=(k == n_mm - 1))
                            k += 1
                    ob = otp.tile([128, NF], F32, name=f"ob{s}_{oi}_{rc}",
                                  tag="ob")
                    nc.scalar.activation(out=ob[:], in_=acc[:], func=AF.Relu,
                                         bias=bn_shift[oi], scale=bn_inv[oi])
                    ob3 = ob.rearrange("p (a b) -> p a b", a=RH)
                    nc.vector.tensor_add(ob3, ob3,
                                         xp[i][oi][:, r0:r0 + RH, 1:W + 1])
                    nc.sync.dma_start(out=y_ap[s, o0:o0 + 128, r0:r0 + RH, :],
                                      in_=ob3)

            prep(0)
            for s in range(BPC):
                if s + 1 < BPC:
                    j = (s + 1) % 2
                    for t in range(CT):
                        nc.sync.dma_start(
                            out=xp[j][t],
                            in_=x_ap[s + 1, t * 128:(t + 1) * 128, :, :])
                conv(s, 0)
                if s + 1 < BPC:
                    prep(s + 1)
                conv(s, 1)

    nc.compile()
    return nc


_CACHE = {}
_LOCK = threading.Lock()


def prepare_in_maps(inputs):
    """Host-side layout prep (sharding + transposes + dtype casts only)."""
    x = np.asarray(inputs["x"], dtype=np.float32)
    route_w = np.asarray(inputs["route_w"], dtype=np.float32)
    route_b = np.ascontiguousarray(np.asarray(inputs["route_b"], dtype=np.float32))
    expert_w = np.asarray(inputs["expert_w"], dtype=np.float32)
    bn_gamma = np.asarray(inputs["bn_gamma"], dtype=np.float32)
    bn_beta = np.asarray(inputs["bn_beta"], dtype=np.float32)
    bn_mean = np.asarray(inputs["bn_mean"], dtype=np.float32)
    bn_var = np.asarray(inputs["bn_var"], dtype=np.float32)

    # [E, COUT, CIN, K, K] -> [E, CIN, K, K, COUT] -> [E, CIN, OTN, KK, 128]
    wt = (expert_w.transpose(0, 2, 3, 4, 1)
          .reshape(E, CIN, KK, OTN, 128)
          .transpose(0, 1, 3, 2, 4))
    wt = np.ascontiguousarray(wt).astype(ml_dtypes.bfloat16)
    rwt = np.ascontiguousarray(route_w.T)  # [CIN, E]
    bnp = np.ascontiguousarray(
        np.stack([bn_gamma, bn_beta, bn_mean, bn_var], axis=1))  # [COUT, 4]

    # width-pad on host: border columns arrive pre-zeroed, so the device DMA
    # is one fully contiguous transfer per (sample, cin-tile)
    xpad = np.zeros((B, CIN, H, WP), dtype=ml_dtypes.bfloat16)
    xpad[:, :, :, 1:W + 1] = x.astype(ml_dtypes.bfloat16)

    return [
        {"x": np.ascontiguousarray(xpad[c * BPC:(c + 1) * BPC]),
         "wt": wt, "rwt": rwt, "rb": route_b, "bnp": bnp}
        for c in range(N_CORES)
    ]


def _get_nc():
    with _LOCK:
        if "nc" not in _CACHE:
            _CACHE["nc"] = build_bass()
        return _CACHE["nc"]


def kernel(**inputs):
    in_maps = prepare_in_maps(inputs)
    nc = _get_nc()
    res = run_bass_kernel_spmd(nc, in_maps, core_ids=list(range(N_CORES)))
    return np.concatenate([r["y"] for r in res.results], axis=0)


# revision 40
# speedup vs baseline: 1.0468x; 1.0468x over previous
"""CondConv (routing -> per-sample mixed 3x3 conv -> frozen BN -> ReLU -> residual)
on 8 Trainium2 NeuronCores, data-parallel over batch (4 samples/core).

Per core:
  - expert bank resident in SBUF as bf16, host-pretransposed to
    [cin, cout-half, kk, 128] so each cout half is contiguous
  - routing: GAP (DVE reduce) -> dot with route_w (DVE + gpsimd partition
    all-reduce; keeps the PE queue free for conv matmuls) -> sigmoid (ACT)
  - per-sample mixed kernel: DVE scalar_tensor_tensor accumulation in bf16,
    split per cout half so the first conv starts after half the mixing
  - conv: per output tile, 18 accumulating bf16 matmuls (2 cin tiles x 3x3
    taps; fp32 PSUM) against width-padded bf16 images; moving dim = 8 rows
    x 56 cols = 448; bf16 weight loads get FWL so LDW hides under the stream
  - BN(frozen)+ReLU fused into the ACT PSUM evacuation, residual add on DVE,
    fp32 output
"""

import threading

import ml_dtypes
import numpy as np

import concourse.bass as bass
import concourse.mybir as mybir
import concourse.tile as tile
from concourse import bacc, bass_isa
from concourse.bass_utils import run_bass_kernel_spmd

F32 = mybir.dt.float32
BF16 = mybir.dt.bfloat16
AX = mybir.AxisListType
OP = mybir.AluOpType
AF = mybir.ActivationFunctionType

N_CORES = 8
B, CIN, COUT, H, W, KS, E = 32, 256, 256, 56, 56, 3, 4
BPC = B // N_CORES  # samples per core
CT = CIN // 128     # cin partition tiles
OTN = COUT // 128   # cout partition tiles
KK = KS * KS
WP = W + 2          # width zero-padded (kj shifts); height handled by clipping
RC = 7              # row chunks per image
RH = H // RC        # rows per chunk
NF = RH * W         # moving-dim elements per matmul
BN_EPS = 1e-5

# conv taps, center first: the center tap covers the full output chunk, so it
# carries start=True and clears every PSUM has_written bit; row-clipped taps
# then accumulate flat sub-slices (= 'same' padding semantics at top/bottom).
TAPS = [(1, 1)] + [(ki, kj) for ki in range(KS) for kj in range(KS)
                   if (ki, kj) != (1, 1)]


def build_bass():
    nc = bacc.Bacc("TRN2", target_bir_lowering=False, debug=False)

    x_d = nc.dram_tensor("x", [BPC, CIN, H, WP], BF16, kind="ExternalInput")
    wt_d = nc.dram_tensor("wt", [E, CIN, OTN, KK, 128], BF16,
                          kind="ExternalInput")
    rwt_d = nc.dram_tensor("rwt", [CIN, E], F32, kind="ExternalInput")
    rb_d = nc.dram_tensor("rb", [E], F32, kind="ExternalInput")
    bnp_d = nc.dram_tensor("bnp", [COUT, 4], F32, kind="ExternalInput")
    y_d = nc.dram_tensor("y", [BPC, COUT, H, W], F32, kind="ExternalOutput")

    x_ap = x_d.ap()
    wt_ap = wt_d.ap()
    rwt_ap = rwt_d.ap()
    rb_ap = rb_d.ap()
    bnp_ap = bnp_d.ap()
    y_ap = y_d.ap()

    with tile.TileContext(nc) as tc:
        with (
            tc.tile_pool(name="wbp", bufs=1) as wbp,
            tc.tile_pool(name="xpp", bufs=1) as xpp,
            tc.tile_pool(name="mwp", bufs=1) as mwp,
            tc.tile_pool(name="otp", bufs=10) as otp,
            tc.tile_pool(name="snp", bufs=1) as snp,
            tc.tile_pool(name="smp", bufs=4) as smp,
            tc.tile_pool(name="psp", bufs=6, space="PSUM") as psp,
        ):
            # ---- persistent tiles ----
            wb = [[wbp.tile([128, OTN, KK, 128], BF16, name=f"wb{e}_{t}",
                            tag=f"wb{e}_{t}")
                   for t in range(CT)] for e in range(E)]
            xp = [[xpp.tile([128, H, WP], BF16, name=f"xp{i}_{t}",
                            tag=f"xp{i}_{t}")
                   for t in range(CT)] for i in range(2)]
            mw = [[mwp.tile([128, OTN, KK, 128], BF16, name=f"mw{i}_{t}",
                            tag=f"mw{i}_{t}")
                   for t in range(CT)] for i in range(2)]
            rwt_sb = [snp.tile([128, E], F32, name=f"rwt{t}", tag=f"rwt{t}")
                      for t in range(CT)]
            rb_bc = snp.tile([128, E], F32, name="rb_bc", tag="rb_bc")
            bn_sb = [snp.tile([128, 4], F32, name=f"bn{o}", tag=f"bn{o}")
                     for o in range(OTN)]
            bn_inv = [snp.tile([128, 1], F32, name=f"bninv{o}", tag=f"bninv{o}")
                      for o in range(OTN)]
            bn_shift = [snp.tile([128, 1], F32, name=f"bnsh{o}", tag=f"bnsh{o}")
                        for o in range(OTN)]

            # ---- preamble DMAs in priority order: queue order = bandwidth
            # priority. x(0) tiles split across two queues (routing critical
            # path), tiny params next, then the expert bank with the oi=0
            # halves first (mixing consumes them first).
            nc.sync.dma_start(out=xp[0][0], in_=x_ap[0, 0:128, :, :])
            nc.scalar.dma_start(out=xp[0][1], in_=x_ap[0, 128:256, :, :])
            for t in range(CT):
                nc.sync.dma_start(out=rwt_sb[t],
                                  in_=rwt_ap[t * 128:(t + 1) * 128, :])
            nc.sync.dma_start(
                out=rb_bc,
                in_=bass.AP(tensor=rb_ap.tensor, offset=0, ap=[[0, 128], [1, E]]))
            for o in range(OTN):
                nc.sync.dma_start(out=bn_sb[o],
                                  in_=bnp_ap[o * 128:(o + 1) * 128, :])
            for oi in range(OTN):
                for e in range(E):
                    for t in range(CT):
                        nc.sync.dma_start(out=wb[e][t][:, oi],
                                          in_=wt_ap[e, t * 128:(t + 1) * 128, oi])

            # all-zeros per-partition scalar: explicit AP bias for ACT funcs
            # (the float-bias path needs a pre-registered const-AP database)
            zeros1 = snp.tile([128, 1], F32, name="zeros1", tag="zeros1")
            nc.vector.memset(zeros1, 0.0)

            # BN folded scale/shift: inv = gamma / sqrt(var+eps);
            # shift = beta - mean * inv
            for o in range(OTN):
                ve = smp.tile([128, 1], F32, name=f"ve{o}", tag="ve")
                nc.vector.tensor_scalar_add(ve, bn_sb[o][:, 3:4], BN_EPS)
                sq = smp.tile([128, 1], F32, name=f"sq{o}", tag="sq")
                nc.scalar.activation(out=sq, in_=ve, func=AF.Sqrt, bias=zeros1)
                nc.vector.reciprocal(out=bn_inv[o], in_=sq)
                nc.vector.tensor_mul(bn_inv[o], bn_inv[o], bn_sb[o][:, 0:1])
                mi = smp.tile([128, 1], F32, name=f"mi{o}", tag="mi")
                nc.vector.tensor_mul(mi, bn_sb[o][:, 2:3], bn_inv[o])
                nc.vector.tensor_sub(bn_shift[o], bn_sb[o][:, 1:2], mi)

            # scratch target for the ACT-side pooled copy (only accum_out used)
            pscr = snp.tile([128, H * W // 2], BF16, name="pscr", tag="pscr")

            # warm-up operands: dependency-gated dummy matmuls keep the PE
            # HAM window busy right before the first real matmul so the real
            # stream starts at full clock (warm_x is touched from `prod` in
            # prep(0) to time the dummies against the routing chain)
            warm_w = snp.tile([128, 128], BF16, name="warm_w", tag="warm_w")
            nc.vector.memset(warm_w, 0.0)
            warm_x = snp.tile([128, NF], BF16, name="warm_x", tag="warm_x")
            nc.vector.memset(warm_x, 0.0)

            def prep(s):
                """Routing + weight mixing for sample s (no PE involvement)."""
                i = s % 2
                pooled = [smp.tile([128, 1], F32, name=f"pool{s}_{t}",
                                   tag=f"pool{t}") for t in range(CT)]
                ph = smp.tile([128, 1], F32, name=f"ph{s}", tag="ph")
                # GAP: tile 0 on DVE; tile 1 split into a DVE half and an ACT
                # (Copy + accum_out) half so its reduction finishes ~2x sooner
                # after the tile-1 DMA lands
                nc.vector.reduce_sum(out=pooled[0], in_=xp[i][0][:, :, 1:W + 1],
                                     axis=AX.XY)
                nc.vector.reduce_sum(out=pooled[1],
                                     in_=xp[i][1][:, 0:H // 2, 1:W + 1],
                                     axis=AX.XY)
                nc.scalar.activation(out=pscr, in_=xp[i][1][:, H // 2:H, 1:W + 1],
                                     func=AF.Copy, accum_out=ph)
                prod = smp.tile([128, E], F32, name=f"prod{s}", tag="prod")
                nc.vector.tensor_scalar_mul(prod, rwt_sb[0], pooled[0])
                nc.vector.scalar_tensor_tensor(out=prod, in0=rwt_sb[1],
                                               scalar=pooled[1], in1=prod,
                                               op0=OP.mult, op1=OP.add)
                nc.vector.scalar_tensor_tensor(out=prod, in0=rwt_sb[1],
                                               scalar=ph, in1=prod,
                                               op0=OP.mult, op1=OP.add)
                if s == 0:
                    # touch warm_x from prod, then issue the warm-up matmuls:
                    # they run while the routing tail + mixing completes
                    nc.vector.tensor_copy(warm_x[0:1, 0:E], prod[0:1, :])
                    wps = psp.tile([128, NF], F32, name="warm_ps",
                                   tag="warmps", bufs=1)
                    for _ in range(18):
                        nc.tensor.matmul(wps[:], lhsT=warm_w, rhs=warm_x,
                                         start=True, stop=True)
                lg = smp.tile([128, E], F32, name=f"lg{s}", tag="lg")
                nc.gpsimd.partition_all_reduce(lg, prod, channels=128,
                                               reduce_op=bass_isa.ReduceOp.add)
                nc.vector.scalar_tensor_tensor(out=lg, in0=lg,
                                               scalar=1.0 / (H * W), in1=rb_bc,
                                               op0=OP.mult, op1=OP.add)
                rr = smp.tile([128, E], F32, name=f"rr{s}", tag="rr")
                nc.scalar.activation(out=rr, in_=lg, func=AF.Sigmoid, bias=zeros1)
                # mix per cout half: the first conv of the sample only waits
                # for the oi=0 half of the bank. cin tile 0 accumulates on
                # DVE; tile 1 gets its expert scaling from ACT (scaled Copy)
                # with DVE doing only the adds, so the two chains overlap.
                for oi in range(OTN):
                    nc.vector.tensor_scalar_mul(mw[i][0][:, oi],
                                                wb[0][0][:, oi], rr[:, 0:1])
                    for e in range(1, E):
                        nc.vector.scalar_tensor_tensor(
                            out=mw[i][0][:, oi], in0=wb[e][0][:, oi],
                            scalar=rr[:, e:e + 1], in1=mw[i][0][:, oi],
                            op0=OP.mult, op1=OP.add)
                    ce = [smp.tile([128, KK, 128], BF16, name=f"ce{s}_{oi}_{e}",
                                   tag=f"ce{e}", bufs=2) for e in range(E)]
                    for e in range(E):
                        nc.scalar.activation(out=ce[e], in_=wb[e][1][:, oi],
                                             func=AF.Copy, scale=rr[:, e:e + 1])
                    nc.vector.tensor_add(mw[i][1][:, oi], ce[0], ce[1])
                    nc.vector.tensor_add(mw[i][1][:, oi], mw[i][1][:, oi], ce[2])
                    nc.vector.tensor_add(mw[i][1][:, oi], mw[i][1][:, oi], ce[3])

            def conv(s, oi):
                """One output channel tile of sample s: matmuls + BN/ReLU +
                residual + store."""
                i = s % 2
                o0 = oi * 128
                n_mm = len(TAPS) * CT
                for rc in range(RC):
                    r0 = rc * RH
                    acc = psp.tile([128, NF], F32, name=f"acc{s}_{oi}_{rc}",
                                   tag="acc")
                    k = 0
                    for t in range(CT):
                        # t-major so a chunk's first 9 matmuls only need the
                        # cin-tile-0 mix chain (shaves the sample-0 start)
                        for ki, kj in TAPS:
                            # valid output rows for this tap (clipped at top/
                            # bottom; kj handled by the zero-padded columns)
                            h_lo = max(r0, 1 - ki)
                            h_hi = min(r0 + RH - 1, H - ki)
                            kki = ki * KS + kj
                            nc.tensor.matmul(
                                acc[:, (h_lo - r0) * W:(h_hi - r0 + 1) * W],
                                lhsT=mw[i][t][:, oi, kki, :],
                                rhs=xp[i][t][:, h_lo + ki - 1:h_hi + ki,
                                             kj:kj + W],
                                start=(k == 0), stop=(k == n_mm - 1))
                            k += 1
                    ob = otp.tile([128, NF], F32, name=f"ob{s}_{oi}_{rc}",
                                  tag="ob")
                    nc.scalar.activation(out=ob[:], in_=acc[:], func=AF.Relu,
                                         bias=bn_shift[oi], scale=bn_inv[oi])
                    ob3 = ob.rearrange("p (a b) -> p a b", a=RH)
                    nc.vector.tensor_add(ob3, ob3,
                                         xp[i][oi][:, r0:r0 + RH, 1:W + 1])
                    nc.sync.dma_start(out=y_ap[s, o0:o0 + 128, r0:r0 + RH, :],
                                      in_=ob3)

            prep(0)
            for s in range(BPC):
                if s + 1 < BPC:
                    j = (s + 1) % 2
                    for t in range(CT):
                        nc.sync.dma_start(
                            out=xp[j][t],
                            in_=x_ap[s + 1, t * 128:(t + 1) * 128, :, :])
                conv(s, 0)
                if s + 1 < BPC:
                    prep(s + 1)
                conv(s, 1)

    nc.compile()
    return nc


_CACHE = {}
_LOCK = threading.Lock()


def prepare_in_maps(inputs):
    """Host-side layout prep (sharding + transposes + dtype casts only)."""
    x = np.asarray(inputs["x"], dtype=np.float32)
    route_w = np.asarray(inputs["route_w"], dtype=np.float32)
    route_b = np.ascontiguousarray(np.asarray(inputs["route_b"], dtype=np.float32))
    expert_w = np.asarray(inputs["expert_w"], dtype=np.float32)
    bn_gamma = np.asarray(inputs["bn_gamma"], dtype=np.float32)
    bn_beta = np.asarray(inputs["bn_beta"], dtype=np.float32)
    bn_mean = np.asarray(inputs["bn_mean"], dtype=np.float32)
    bn_var = np.asarray(inputs["bn_var"], dtype=np.float32)

    # [E, COUT, CIN, K, K] -> [E, CIN, K, K, COUT] -> [E, CIN, OTN, KK, 128]
    wt = (expert_w.transpose(0, 2, 3, 4, 1)
          .reshape(E, CIN, KK, OTN, 128)
          .transpose(0, 1, 3, 2, 4))
    wt = np.ascontiguousarray(wt).astype(ml_dtypes.bfloat16)
    rwt = np.ascontiguousarray(route_w.T)  # [CIN, E]
    bnp = np.ascontiguousarray(
        np.stack([bn_gamma, bn_beta, bn_mean, bn_var], axis=1))  # [COUT, 4]

    # width-pad on host: border columns arrive pre-zeroed, so the device DMA
    # is one fully contiguous transfer per (sample, cin-tile)
    xpad = np.zeros((B, CIN, H, WP), dtype=ml_dtypes.bfloat16)
    xpad[:, :, :, 1:W + 1] = x.astype(ml_dtypes.bfloat16)

    return [
        {"x": np.ascontiguousarray(xpad[c * BPC:(c + 1) * BPC]),
         "wt": wt, "rwt": rwt, "rb": route_b, "bnp": bnp}
        for c in range(N_CORES)
    ]


def _get_nc():
    with _LOCK:
        if "nc" not in _CACHE:
            _CACHE["nc"] = build_bass()
        return _CACHE["nc"]


def kernel(**inputs):
    in_maps = prepare_in_maps(inputs)
    nc = _get_nc()
    res = run_bass_kernel_spmd(nc, in_maps, core_ids=list(range(N_CORES)))
    return np.concatenate([r["y"] for r in res.results], axis=0)
